# revision 15
# baseline (speedup 1.0000x reference)
"""Trainium2 Bass kernel for AdaptiveLiquidNetwork.

Reference computation (per full batch B=16384):
    projected  = tanh(x @ w_in.T + b_in)                     [B, U]
    A          = sigmoid(projected @ sensory_w + sigma)      [B, U]
    decay      = exp(-0.1 / tau)                             [U]
    new_states = A + (states - A) * decay                    [B, U]
    output     = new_states @ readout_w.T + readout_b        [B, D]

Strategy: data-parallel over 8 NeuronCores (2048 batch rows each),
weights replicated. On-chip dataflow keeps activations feature-major
([feature_part, batch_free]) so the contraction dim always sits on the
SBUF partition axis:
  - x is cast f32->bf16 during the DMA load (SWDGE), then transposed on
    the TensorEngine (128x128 identity transposes) into xT [d_in, b].
    The transposes are interleaved with mm1's accumulation (kc-outer
    loop, 4 live PSUM groups) so the PE HAM clock gate stays warm.
  - mm1: projT[u, b] = w_inT[k, u] slices (stationary) x xT[k, b];
    tanh+bias fused into the ScalarEngine PSUM->SBUF activation
    (bias is per-partition in this orientation).
  - mm2: A_T[u', b] = sensory_w[u, u'] (natural layout!) x projT[u, b],
    sigmoid+bias fused the same way.
  - new_states: A_T is transposed back 128x128 at a time; the
    PSUM->SBUF copy doubles as the "*(1-decay)" multiply (zero-states
    fast path) via a broadcast tile on the VectorEngine.
  - mm3 is flipped: lhsT = A_T 128-column slices (already feature
    major), rhs = readout_w.T (pre-scaled by (1-decay) on host for the
    zero-states path) -> output lands batch-major in PSUM directly; the
    readout bias is added by the VectorEngine during the PSUM->SBUF
    copy with a broadcast bias tile.

All matmuls run in bf16 (fp32 PSUM accumulation): on TRN2 bf16 streams
one column/cycle like fp32r, but LDWEIGHTS gets FWL + background-buffer
pull-ahead, which f32/f32r cannot use (their ~150ns weight load
serializes with every matmul).

Host-side prep is limited to weight re-layout/casting (transposes of
the small replicated [512,*] matrices, per-partition vector packing)
and the exp(-t/tau) scalar math; all O(B) work happens on-device.
"""

import os
import sys

import numpy as np

for _p in (
    "/opt/trn_rl_repo",
    os.path.expanduser("~/.axon_site"),
    os.path.expanduser("~/.axon_site/_ro/trn_rl_repo"),
    os.path.expanduser("~/.axon_site/_ro/pypackages"),
):
    if os.path.isdir(_p) and _p not in sys.path:
        sys.path.append(_p)

import ml_dtypes  # noqa: E402

import concourse.bass as bass  # noqa: E402
import concourse.tile as tile  # noqa: E402
from concourse import bacc, mybir  # noqa: E402
from concourse.bass_utils import run_bass_kernel_spmd  # noqa: E402

F32 = mybir.dt.float32
BF16 = mybir.dt.bfloat16
AF = mybir.ActivationFunctionType
NP_BF16 = ml_dtypes.bfloat16

N_CORES = 8
B = 16384
D_IN = 1024
U = 512
D_OUT = 512
T_END = 0.1

BS = B // N_CORES          # batch rows per core (2048)
BCHUNK = 512               # batch rows per processing chunk
N_BC = BS // BCHUNK        # chunks per core (4)
N_BSUB = BCHUNK // 128     # 128-row subtiles per chunk (4)
N_KC = D_IN // 128         # contraction tiles for mm1 (8)
N_UC = U // 128            # feature tiles (4)


def _build(with_states: bool):
    nc = bacc.Bacc("TRN2", target_bir_lowering=False, debug=False)

    x = nc.dram_tensor("x", [BS, D_IN], F32, kind="ExternalInput").ap()
    w_inT = nc.dram_tensor("w_inT", [D_IN, U], BF16, kind="ExternalInput").ap()
    sensory = nc.dram_tensor("sensory_w", [U, U], BF16, kind="ExternalInput").ap()
    readout_wT = nc.dram_tensor("readout_wT", [U, D_OUT], BF16, kind="ExternalInput").ap()
    # vecs columns: [0:4]=b_in, [4:8]=sigma, [8:12]=1-decay, [12:16]=decay,
    # each packed [128, 4] with element [p, c] = v[c*128 + p].
    vecs = nc.dram_tensor("vecs", [128, 16], F32, kind="ExternalInput").ap()
    rb_bcast = nc.dram_tensor("rb_bcast", [128, D_OUT], F32, kind="ExternalInput").ap()
    omd_bcast = nc.dram_tensor("omd_bcast", [128, U], BF16, kind="ExternalInput").ap()
    eye = nc.dram_tensor("eye128", [128, 128], BF16, kind="ExternalInput").ap()
    states = None
    if with_states:
        states = nc.dram_tensor("states", [BS, U], F32, kind="ExternalInput").ap()

    out = nc.dram_tensor("out", [BS, D_OUT], F32, kind="ExternalOutput").ap()
    new_states = nc.dram_tensor("new_states", [BS, U], F32, kind="ExternalOutput").ap()

    with tile.TileContext(nc) as tc:
        with (
            tc.tile_pool(name="const", bufs=1) as cpool,
            tc.tile_pool(name="xin", bufs=10) as xpool,
            tc.tile_pool(name="xt", bufs=4) as xtpool,
            tc.tile_pool(name="act", bufs=6) as apool,
            tc.tile_pool(name="onat", bufs=4) as opool,
            tc.tile_pool(name="pst", bufs=2, space="PSUM") as trppool,
            tc.tile_pool(name="psmm", bufs=4, space="PSUM") as mmppool,
            tc.tile_pool(name="psmm3", bufs=2, space="PSUM") as mm3ppool,
        ):
            # ---- constants / weights, loaded once ----
            w_sb = cpool.tile([128, N_KC * U], BF16, tag="w_in")
            nc.sync.dma_start(
                out=w_sb[:].rearrange("p (kc u) -> p kc u", kc=N_KC),
                in_=w_inT.rearrange("(kc p) u -> p kc u", p=128),
            )
            ss_sb = cpool.tile([128, N_UC * U], BF16, tag="sensory")
            nc.sync.dma_start(
                out=ss_sb[:].rearrange("p (uc u) -> p uc u", uc=N_UC),
                in_=sensory.rearrange("(uc p) u -> p uc u", p=128),
            )
            rt_sb = cpool.tile([128, N_UC * D_OUT], BF16, tag="readout")
            nc.sync.dma_start(
                out=rt_sb[:].rearrange("p (uc d) -> p uc d", uc=N_UC),
                in_=readout_wT.rearrange("(uc p) d -> p uc d", p=128),
            )
            vec_sb = cpool.tile([128, 16], F32, tag="vecs")
            nc.sync.dma_start(out=vec_sb[:], in_=vecs[:])
            rb_sb = cpool.tile([128, D_OUT], F32, tag="rb")
            nc.sync.dma_start(out=rb_sb[:], in_=rb_bcast[:])
            omd_sb = cpool.tile([128, U], BF16, tag="omd")
            nc.sync.dma_start(out=omd_sb[:], in_=omd_bcast[:])
            eye_sb = cpool.tile([128, 128], BF16, tag="eye")
            nc.sync.dma_start(out=eye_sb[:], in_=eye[:])

            def load_x(bc):
                row0 = bc * BCHUNK
                xa = []
                for i in range(N_BSUB):
                    t = xpool.tile([128, D_IN], BF16, tag="xa")
                    # SWDGE cast DMA: f32 DRAM -> bf16 SBUF
                    nc.gpsimd.dma_start(
                        out=t[:], in_=x[row0 + i * 128 : row0 + (i + 1) * 128, :]
                    )
                    xa.append(t)
                return xa

            xa_next = load_x(0)
            for bc in range(N_BC):
                row0 = bc * BCHUNK
                xa = xa_next
                if bc + 1 < N_BC:
                    xa_next = load_x(bc + 1)

                # ---- mm1, kc-outer so transposes interleave with matmuls
                # (keeps the PE HAM clock-gate warm): for each k-chunk,
                # transpose 4 x subtiles into one PSUM bank, copy to SBUF
                # (ScalarEngine), then immediately accumulate that k-chunk
                # into all 4 uc PSUM groups. ----
                ps1 = [
                    mmppool.tile([128, BCHUNK], F32, tag="mm", name=f"ps1_{bc}_{uc}")
                    for uc in range(N_UC)
                ]
                for kc in range(N_KC):
                    pt = trppool.tile([128, BCHUNK], BF16, tag="tr")
                    for i in range(N_BSUB):
                        nc.tensor.transpose(
                            pt[:, i * 128 : (i + 1) * 128],
                            xa[i][:, kc * 128 : (kc + 1) * 128],
                            eye_sb[:],
                        )
                    xt = xtpool.tile([128, BCHUNK], BF16, tag="xt")
                    nc.scalar.activation(xt[:], pt[:], AF.Copy)
                    for uc in range(N_UC):
                        nc.tensor.matmul(
                            ps1[uc][:],
                            lhsT=w_sb[:, kc * U + uc * 128 : kc * U + (uc + 1) * 128],
                            rhs=xt[:],
                            start=(kc == 0),
                            stop=(kc == N_KC - 1),
                        )

                projT = []
                for uc in range(N_UC):
                    t = apool.tile([128, BCHUNK], BF16, tag="projT")
                    nc.scalar.activation(
                        t[:], ps1[uc][:], AF.Tanh, bias=vec_sb[:, uc : uc + 1]
                    )
                    projT.append(t)

                # ---- mm2 + sigmoid -> A_T[uc2] [128u', 512b] ----
                A_T = []
                for uc2 in range(N_UC):
                    ps = mmppool.tile([128, BCHUNK], F32, tag="mm")
                    for uc in range(N_UC):
                        nc.tensor.matmul(
                            ps[:],
                            lhsT=ss_sb[:, uc * U + uc2 * 128 : uc * U + (uc2 + 1) * 128],
                            rhs=projT[uc][:],
                            start=(uc == 0),
                            stop=(uc == N_UC - 1),
                        )
                    t = apool.tile([128, BCHUNK], BF16, tag="A_T")
                    nc.scalar.activation(
                        t[:], ps[:], AF.Sigmoid, bias=vec_sb[:, 4 + uc2 : 5 + uc2]
                    )
                    A_T.append(t)

                if not with_states:
                    # new_states = A * (1-decay); mm3 consumes A_T directly
                    # (readout_wT pre-scaled by (1-decay) on host).
                    nsT = A_T
                else:
                    # general path: new_states = A*(1-decay) + states*decay
                    st_nat = []
                    for i in range(N_BSUB):
                        t = xpool.tile([128, U], BF16, tag="st_nat", bufs=6)
                        # SWDGE cast DMA: f32 DRAM -> bf16 SBUF
                        nc.gpsimd.dma_start(
                            out=t[:],
                            in_=states[row0 + i * 128 : row0 + (i + 1) * 128, :],
                        )
                        st_nat.append(t)
                    nsT = []
                    for uc2 in range(N_UC):
                        pt = trppool.tile([128, BCHUNK], BF16, tag="tr")
                        for i in range(N_BSUB):
                            nc.tensor.transpose(
                                pt[:, i * 128 : (i + 1) * 128],
                                st_nat[i][:, uc2 * 128 : (uc2 + 1) * 128],
                                eye_sb[:],
                            )
                        stT = xtpool.tile([128, BCHUNK], F32, tag="stT", bufs=2)
                        nc.vector.tensor_copy(stT[:], pt[:])
                        t1 = apool.tile([128, BCHUNK], F32, tag="ns_a", bufs=2)
                        nc.vector.tensor_scalar_mul(
                            t1[:], A_T[uc2][:], vec_sb[:, 8 + uc2 : 9 + uc2]
                        )
                        t2 = apool.tile([128, BCHUNK], F32, tag="ns_s", bufs=2)
                        nc.vector.tensor_scalar_mul(
                            t2[:], stT[:], vec_sb[:, 12 + uc2 : 13 + uc2]
                        )
                        t3 = apool.tile([128, BCHUNK], BF16, tag="nsT", bufs=6)
                        nc.vector.tensor_add(t3[:], t1[:], t2[:])
                        nsT.append(t3)

                # ---- new_states back-transpose + DMA out ----
                for i in range(N_BSUB):
                    pt = trppool.tile([128, U], BF16, tag="tr")
                    for uc2 in range(N_UC):
                        nc.tensor.transpose(
                            pt[:, uc2 * 128 : (uc2 + 1) * 128],
                            nsT[uc2][:, i * 128 : (i + 1) * 128],
                            eye_sb[:],
                        )
                    nsn = opool.tile([128, U], F32, tag="ns_nat")
                    if with_states:
                        nc.vector.tensor_copy(nsn[:], pt[:])
                    else:
                        # fuse the *(1-decay) into the PSUM->SBUF copy
                        nc.vector.tensor_mul(nsn[:], pt[:], omd_sb[:])
                    nc.sync.dma_start(
                        out=new_states[row0 + i * 128 : row0 + (i + 1) * 128, :],
                        in_=nsn[:],
                    )

                # ---- mm3 (batch-major output) + bias + DMA out ----
                for i in range(N_BSUB):
                    ps = mm3ppool.tile([128, D_OUT], F32, tag="mm3")
                    for uc2 in range(N_UC):
                        nc.tensor.matmul(
                            ps[:],
                            lhsT=nsT[uc2][:, i * 128 : (i + 1) * 128],
                            rhs=rt_sb[:, uc2 * D_OUT : (uc2 + 1) * D_OUT],
                            start=(uc2 == 0),
                            stop=(uc2 == N_UC - 1),
                        )
                    ob = opool.tile([128, D_OUT], F32, tag="ob")
                    nc.vector.tensor_add(ob[:], ps[:], rb_sb[:])
                    nc.sync.dma_start(
                        out=out[row0 + i * 128 : row0 + (i + 1) * 128, :], in_=ob[:]
                    )

    nc.compile()
    return nc


_GRAPHS: dict[bool, object] = {}


def _get_graph(with_states: bool):
    if with_states not in _GRAPHS:
        _GRAPHS[with_states] = _build(with_states)
    return _GRAPHS[with_states]


def _pack_cols(v):
    """[512] -> [128, 4] with [p, c] = v[c*128 + p]."""
    return np.ascontiguousarray(np.asarray(v, np.float32).reshape(4, 128).T)


def kernel(
    x,
    w_in,
    b_in,
    sensory_w,
    sensory_sigma,
    tau,
    readout_w,
    readout_b,
    states,
    _profile=False,
):
    x = np.ascontiguousarray(np.asarray(x, np.float32))
    w_in = np.asarray(w_in, np.float32)
    b_in = np.asarray(b_in, np.float32)
    sensory_w = np.asarray(sensory_w, np.float32)
    sensory_sigma = np.asarray(sensory_sigma, np.float32)
    tau = np.asarray(tau, np.float32)
    readout_w = np.asarray(readout_w, np.float32)
    readout_b = np.asarray(readout_b, np.float32)
    states = np.ascontiguousarray(np.asarray(states, np.float32))

    decay = np.exp(-T_END / tau).astype(np.float32)
    omd = (1.0 - decay).astype(np.float32)
    with_states = bool(states.any())

    w_inT = np.ascontiguousarray(w_in.T.astype(NP_BF16))
    rwT = readout_w.T.astype(np.float32)
    if not with_states:
        rwT = rwT * omd[:, None]
    readout_wT = np.ascontiguousarray(rwT.astype(NP_BF16))

    vecs = np.concatenate(
        [_pack_cols(b_in), _pack_cols(sensory_sigma), _pack_cols(omd), _pack_cols(decay)],
        axis=1,
    ).astype(np.float32)
    rb_bcast = np.ascontiguousarray(
        np.broadcast_to(readout_b, (128, D_OUT)).astype(np.float32)
    )
    omd_bcast = np.ascontiguousarray(np.broadcast_to(omd, (128, U)).astype(NP_BF16))
    eye = np.eye(128, dtype=NP_BF16)

    nc = _get_graph(with_states)

    in_maps = []
    for c in range(N_CORES):
        m = {
            "x": x[c * BS : (c + 1) * BS],
            "w_inT": w_inT,
            "sensory_w": np.ascontiguousarray(sensory_w.astype(NP_BF16)),
            "readout_wT": readout_wT,
            "vecs": vecs,
            "rb_bcast": rb_bcast,
            "omd_bcast": omd_bcast,
            "eye128": eye,
        }
        if with_states:
            m["states"] = states[c * BS : (c + 1) * BS]
        in_maps.append(m)

    res = run_bass_kernel_spmd(
        nc, in_maps, core_ids=list(range(N_CORES)), trace=_profile
    )

    out = np.concatenate([res.results[c]["out"] for c in range(N_CORES)], axis=0)
    new_states = np.concatenate(
        [res.results[c]["new_states"] for c in range(N_CORES)], axis=0
    )
    if _profile:
        return (out, new_states), res
    return (out, new_states)


# revision 21
# speedup vs baseline: 1.2222x; 1.2222x over previous
"""Trainium2 Bass kernel for AdaptiveLiquidNetwork.

Reference computation (per full batch B=16384):
    projected  = tanh(x @ w_in.T + b_in)                     [B, U]
    A          = sigmoid(projected @ sensory_w + sigma)      [B, U]
    decay      = exp(-0.1 / tau)                             [U]
    new_states = A + (states - A) * decay                    [B, U]
    output     = new_states @ readout_w.T + readout_b        [B, D]

Strategy: data-parallel over 8 NeuronCores (2048 batch rows each),
weights replicated. On-chip dataflow keeps activations feature-major
([feature_part, batch_free]) so the contraction dim always sits on the
SBUF partition axis:
  - x is cast f32->bf16 during the DMA load (SWDGE), then transposed on
    the TensorEngine (128x128 identity transposes) into xT [d_in, b].
    The transposes are interleaved with mm1's accumulation (kc-outer
    loop, 4 live PSUM groups) so the PE HAM clock gate stays warm.
  - mm1: projT[u, b] = w_inT[k, u] slices (stationary) x xT[k, b];
    tanh+bias fused into the ScalarEngine PSUM->SBUF activation
    (bias is per-partition in this orientation).
  - mm2: A_T[u', b] = sensory_w[u, u'] (natural layout!) x projT[u, b],
    sigmoid+bias fused the same way.
  - new_states: A_T is transposed back 128x128 at a time; the
    PSUM->SBUF copy doubles as the "*(1-decay)" multiply (zero-states
    fast path) via a broadcast tile on the VectorEngine.
  - mm3 is flipped: lhsT = A_T 128-column slices (already feature
    major), rhs = readout_w.T (pre-scaled by (1-decay) on host for the
    zero-states path) -> output lands batch-major in PSUM directly; the
    readout bias is added by the VectorEngine during the PSUM->SBUF
    copy with a broadcast bias tile.

All matmuls run in bf16 (fp32 PSUM accumulation): on TRN2 bf16 streams
one column/cycle like fp32r, but LDWEIGHTS gets FWL + background-buffer
pull-ahead, which f32/f32r cannot use (their ~150ns weight load
serializes with every matmul).

Host-side prep is limited to weight re-layout/casting (transposes of
the small replicated [512,*] matrices, per-partition vector packing)
and the exp(-t/tau) scalar math; all O(B) work happens on-device.
"""

import os
import sys

import numpy as np

for _p in (
    "/opt/trn_rl_repo",
    os.path.expanduser("~/.axon_site"),
    os.path.expanduser("~/.axon_site/_ro/trn_rl_repo"),
    os.path.expanduser("~/.axon_site/_ro/pypackages"),
):
    if os.path.isdir(_p) and _p not in sys.path:
        sys.path.append(_p)

import ml_dtypes  # noqa: E402

import concourse.bass as bass  # noqa: E402
import concourse.tile as tile  # noqa: E402
from concourse import bacc, mybir  # noqa: E402
from concourse.bass_utils import run_bass_kernel_spmd  # noqa: E402

F32 = mybir.dt.float32
BF16 = mybir.dt.bfloat16
AF = mybir.ActivationFunctionType
NP_BF16 = ml_dtypes.bfloat16

N_CORES = 8
B = 16384
D_IN = 1024
U = 512
D_OUT = 512
T_END = 0.1

BS = B // N_CORES          # batch rows per core (2048)
BCHUNK = 512               # batch rows per processing chunk
N_BC = BS // BCHUNK        # chunks per core (4)
N_BSUB = BCHUNK // 128     # 128-row subtiles per chunk (4)
N_KC = D_IN // 128         # contraction tiles for mm1 (8)
N_UC = U // 128            # feature tiles (4)


def _build(with_states: bool):
    nc = bacc.Bacc("TRN2", target_bir_lowering=False, debug=False)

    x = nc.dram_tensor("x", [BS, D_IN], F32, kind="ExternalInput").ap()
    w_inT = nc.dram_tensor("w_inT", [D_IN, U], BF16, kind="ExternalInput").ap()
    sensory = nc.dram_tensor("sensory_w", [U, U], BF16, kind="ExternalInput").ap()
    readout_wT = nc.dram_tensor("readout_wT", [U, D_OUT], BF16, kind="ExternalInput").ap()
    # vecs columns: [0:4]=b_in, [4:8]=sigma, [8:12]=1-decay, [12:16]=decay,
    # each packed [128, 4] with element [p, c] = v[c*128 + p].
    vecs = nc.dram_tensor("vecs", [128, 16], F32, kind="ExternalInput").ap()
    rb_bcast = nc.dram_tensor("rb_bcast", [128, D_OUT], F32, kind="ExternalInput").ap()
    omd_bcast = nc.dram_tensor("omd_bcast", [128, U], BF16, kind="ExternalInput").ap()
    eye = nc.dram_tensor("eye128", [128, 128], BF16, kind="ExternalInput").ap()
    eye32 = nc.dram_tensor("eye128f", [128, 128], F32, kind="ExternalInput").ap()
    states = None
    if with_states:
        states = nc.dram_tensor("states", [BS, U], F32, kind="ExternalInput").ap()

    out = nc.dram_tensor("out", [BS, D_OUT], F32, kind="ExternalOutput").ap()
    new_states = nc.dram_tensor("new_states", [BS, U], F32, kind="ExternalOutput").ap()

    with tile.TileContext(nc) as tc:
        with (
            tc.tile_pool(name="const", bufs=1) as cpool,
            tc.tile_pool(name="xin", bufs=10) as xpool,
            tc.tile_pool(name="xt", bufs=4) as xtpool,
            tc.tile_pool(name="act", bufs=6) as apool,
            tc.tile_pool(name="onat", bufs=4) as opool,
            tc.tile_pool(name="pst", bufs=2, space="PSUM") as trppool,
            tc.tile_pool(name="psmm", bufs=4, space="PSUM") as mmppool,
            tc.tile_pool(name="psmm3", bufs=2, space="PSUM") as mm3ppool,
        ):
            # ---- constants / weights, loaded once ----
            w_sb = cpool.tile([128, N_KC * U], BF16, tag="w_in")
            nc.sync.dma_start(
                out=w_sb[:].rearrange("p (kc u) -> p kc u", kc=N_KC),
                in_=w_inT.rearrange("(kc p) u -> p kc u", p=128),
            )
            ss_sb = cpool.tile([128, N_UC * U], BF16, tag="sensory")
            nc.sync.dma_start(
                out=ss_sb[:].rearrange("p (uc u) -> p uc u", uc=N_UC),
                in_=sensory.rearrange("(uc p) u -> p uc u", p=128),
            )
            rt_sb = cpool.tile([128, N_UC * D_OUT], BF16, tag="readout")
            nc.sync.dma_start(
                out=rt_sb[:].rearrange("p (uc d) -> p uc d", uc=N_UC),
                in_=readout_wT.rearrange("(uc p) d -> p uc d", p=128),
            )
            vec_sb = cpool.tile([128, 16], F32, tag="vecs")
            nc.sync.dma_start(out=vec_sb[:], in_=vecs[:])
            rb_sb = cpool.tile([128, D_OUT], F32, tag="rb")
            nc.sync.dma_start(out=rb_sb[:], in_=rb_bcast[:])
            omd_sb = cpool.tile([128, U], BF16, tag="omd")
            nc.sync.dma_start(out=omd_sb[:], in_=omd_bcast[:])
            eye_sb = cpool.tile([128, 128], BF16, tag="eye")
            nc.sync.dma_start(out=eye_sb[:], in_=eye[:])
            eye32_sb = cpool.tile([128, 128], F32, tag="eye32")
            nc.sync.dma_start(out=eye32_sb[:], in_=eye32[:])

            def load_x(bc):
                row0 = bc * BCHUNK
                xa = []
                for i in range(N_BSUB):
                    t = xpool.tile([128, D_IN], F32, tag="xa")
                    nc.sync.dma_start(
                        out=t[:], in_=x[row0 + i * 128 : row0 + (i + 1) * 128, :]
                    )
                    xa.append(t)
                return xa

            xa_next = load_x(0)
            for bc in range(N_BC):
                row0 = bc * BCHUNK
                xa = xa_next
                if bc + 1 < N_BC:
                    xa_next = load_x(bc + 1)

                # ---- mm1, kc-outer so transposes interleave with matmuls
                # (keeps the PE HAM clock-gate warm): for each k-chunk,
                # transpose 4 x subtiles into one PSUM bank, copy to SBUF
                # (ScalarEngine), then immediately accumulate that k-chunk
                # into all 4 uc PSUM groups. ----
                ps1 = [
                    mmppool.tile([128, BCHUNK], F32, tag="mm", name=f"ps1_{bc}_{uc}")
                    for uc in range(N_UC)
                ]
                for kc in range(N_KC):
                    pt = trppool.tile([128, BCHUNK], F32, tag="tr")
                    for i in range(N_BSUB):
                        nc.tensor.transpose(
                            pt[:, i * 128 : (i + 1) * 128],
                            xa[i][:, kc * 128 : (kc + 1) * 128],
                            eye32_sb[:],
                        )
                    xt = xtpool.tile([128, BCHUNK], BF16, tag="xt")
                    # PSUM->SBUF copy doubles as the f32->bf16 cast; alternate
                    # engines so neither ScalarE nor VectorE serializes mm1
                    if kc % 2 == 0:
                        nc.scalar.activation(xt[:], pt[:], AF.Copy)
                    else:
                        nc.vector.tensor_copy(xt[:], pt[:])
                    for uc in range(N_UC):
                        nc.tensor.matmul(
                            ps1[uc][:],
                            lhsT=w_sb[:, kc * U + uc * 128 : kc * U + (uc + 1) * 128],
                            rhs=xt[:],
                            start=(kc == 0),
                            stop=(kc == N_KC - 1),
                        )

                projT = []
                for uc in range(N_UC):
                    t = apool.tile([128, BCHUNK], BF16, tag="projT")
                    nc.scalar.activation(
                        t[:], ps1[uc][:], AF.Tanh, bias=vec_sb[:, uc : uc + 1]
                    )
                    projT.append(t)

                # ---- mm2 + sigmoid -> A_T[uc2] [128u', 512b] ----
                A_T = []
                for uc2 in range(N_UC):
                    ps = mmppool.tile([128, BCHUNK], F32, tag="mm")
                    for uc in range(N_UC):
                        nc.tensor.matmul(
                            ps[:],
                            lhsT=ss_sb[:, uc * U + uc2 * 128 : uc * U + (uc2 + 1) * 128],
                            rhs=projT[uc][:],
                            start=(uc == 0),
                            stop=(uc == N_UC - 1),
                        )
                    t = apool.tile([128, BCHUNK], BF16, tag="A_T")
                    nc.scalar.activation(
                        t[:], ps[:], AF.Sigmoid, bias=vec_sb[:, 4 + uc2 : 5 + uc2]
                    )
                    A_T.append(t)

                if not with_states:
                    # new_states = A * (1-decay); mm3 consumes A_T directly
                    # (readout_wT pre-scaled by (1-decay) on host).
                    nsT = A_T
                else:
                    # general path: new_states = A*(1-decay) + states*decay
                    st_nat = []
                    for i in range(N_BSUB):
                        t = xpool.tile([128, U], F32, tag="st_nat", bufs=6)
                        nc.sync.dma_start(
                            out=t[:],
                            in_=states[row0 + i * 128 : row0 + (i + 1) * 128, :],
                        )
                        st_nat.append(t)
                    nsT = []
                    for uc2 in range(N_UC):
                        pt = trppool.tile([128, BCHUNK], F32, tag="tr")
                        for i in range(N_BSUB):
                            nc.tensor.transpose(
                                pt[:, i * 128 : (i + 1) * 128],
                                st_nat[i][:, uc2 * 128 : (uc2 + 1) * 128],
                                eye32_sb[:],
                            )
                        stT = xtpool.tile([128, BCHUNK], F32, tag="stT", bufs=2)
                        nc.vector.tensor_copy(stT[:], pt[:])
                        t1 = apool.tile([128, BCHUNK], F32, tag="ns_a", bufs=2)
                        nc.vector.tensor_scalar_mul(
                            t1[:], A_T[uc2][:], vec_sb[:, 8 + uc2 : 9 + uc2]
                        )
                        t2 = apool.tile([128, BCHUNK], F32, tag="ns_s", bufs=2)
                        nc.vector.tensor_scalar_mul(
                            t2[:], stT[:], vec_sb[:, 12 + uc2 : 13 + uc2]
                        )
                        t3 = apool.tile([128, BCHUNK], BF16, tag="nsT", bufs=6)
                        nc.vector.tensor_add(t3[:], t1[:], t2[:])
                        nsT.append(t3)

                # ---- new_states back-transpose + DMA out ----
                for i in range(N_BSUB):
                    pt = trppool.tile([128, U], BF16, tag="tr")
                    for uc2 in range(N_UC):
                        nc.tensor.transpose(
                            pt[:, uc2 * 128 : (uc2 + 1) * 128],
                            nsT[uc2][:, i * 128 : (i + 1) * 128],
                            eye_sb[:],
                        )
                    nsn = opool.tile([128, U], F32, tag="ns_nat")
                    if with_states:
                        nc.vector.tensor_copy(nsn[:], pt[:])
                    else:
                        # fuse the *(1-decay) into the PSUM->SBUF copy
                        nc.vector.tensor_mul(nsn[:], pt[:], omd_sb[:])
                    nc.sync.dma_start(
                        out=new_states[row0 + i * 128 : row0 + (i + 1) * 128, :],
                        in_=nsn[:],
                    )

                # ---- mm3 (batch-major output) + bias + DMA out ----
                for i in range(N_BSUB):
                    ps = mm3ppool.tile([128, D_OUT], F32, tag="mm3")
                    for uc2 in range(N_UC):
                        nc.tensor.matmul(
                            ps[:],
                            lhsT=nsT[uc2][:, i * 128 : (i + 1) * 128],
                            rhs=rt_sb[:, uc2 * D_OUT : (uc2 + 1) * D_OUT],
                            start=(uc2 == 0),
                            stop=(uc2 == N_UC - 1),
                        )
                    ob = opool.tile([128, D_OUT], F32, tag="ob")
                    nc.vector.tensor_add(ob[:], ps[:], rb_sb[:])
                    nc.sync.dma_start(
                        out=out[row0 + i * 128 : row0 + (i + 1) * 128, :], in_=ob[:]
                    )

    nc.compile()
    return nc


_GRAPHS: dict[bool, object] = {}


def _get_graph(with_states: bool):
    if with_states not in _GRAPHS:
        _GRAPHS[with_states] = _build(with_states)
    return _GRAPHS[with_states]


def _pack_cols(v):
    """[512] -> [128, 4] with [p, c] = v[c*128 + p]."""
    return np.ascontiguousarray(np.asarray(v, np.float32).reshape(4, 128).T)


def kernel(
    x,
    w_in,
    b_in,
    sensory_w,
    sensory_sigma,
    tau,
    readout_w,
    readout_b,
    states,
    _profile=False,
):
    x = np.ascontiguousarray(np.asarray(x, np.float32))
    w_in = np.asarray(w_in, np.float32)
    b_in = np.asarray(b_in, np.float32)
    sensory_w = np.asarray(sensory_w, np.float32)
    sensory_sigma = np.asarray(sensory_sigma, np.float32)
    tau = np.asarray(tau, np.float32)
    readout_w = np.asarray(readout_w, np.float32)
    readout_b = np.asarray(readout_b, np.float32)
    states = np.ascontiguousarray(np.asarray(states, np.float32))

    decay = np.exp(-T_END / tau).astype(np.float32)
    omd = (1.0 - decay).astype(np.float32)
    with_states = bool(states.any())

    w_inT = np.ascontiguousarray(w_in.T.astype(NP_BF16))
    rwT = readout_w.T.astype(np.float32)
    if not with_states:
        rwT = rwT * omd[:, None]
    readout_wT = np.ascontiguousarray(rwT.astype(NP_BF16))

    vecs = np.concatenate(
        [_pack_cols(b_in), _pack_cols(sensory_sigma), _pack_cols(omd), _pack_cols(decay)],
        axis=1,
    ).astype(np.float32)
    rb_bcast = np.ascontiguousarray(
        np.broadcast_to(readout_b, (128, D_OUT)).astype(np.float32)
    )
    omd_bcast = np.ascontiguousarray(np.broadcast_to(omd, (128, U)).astype(NP_BF16))
    eye = np.eye(128, dtype=NP_BF16)
    eye32 = np.eye(128, dtype=np.float32)

    nc = _get_graph(with_states)

    in_maps = []
    for c in range(N_CORES):
        m = {
            "x": x[c * BS : (c + 1) * BS],
            "w_inT": w_inT,
            "sensory_w": np.ascontiguousarray(sensory_w.astype(NP_BF16)),
            "readout_wT": readout_wT,
            "vecs": vecs,
            "rb_bcast": rb_bcast,
            "omd_bcast": omd_bcast,
            "eye128": eye,
            "eye128f": eye32,
        }
        if with_states:
            m["states"] = states[c * BS : (c + 1) * BS]
        in_maps.append(m)

    res = run_bass_kernel_spmd(
        nc, in_maps, core_ids=list(range(N_CORES)), trace=_profile
    )

    out = np.concatenate([res.results[c]["out"] for c in range(N_CORES)], axis=0)
    new_states = np.concatenate(
        [res.results[c]["new_states"] for c in range(N_CORES)], axis=0
    )
    if _profile:
        return (out, new_states), res
    return (out, new_states)
